# revision 40
# baseline (speedup 1.0000x reference)
"""Trainium2 Bass kernel for nn_ContextEmbedding (cross-attention context embedding).

Reference math (per batch b):
    Q = E @ q_w.T        [256, 1024]
    K = I @ k_w.T        [4096, 1024]
    V = I @ v_w.T        [4096, 1024]
    S_h = Q_h @ K_h.T    per head (16 heads, head_dim 64)
    P = softmax(S, -1)
    U_h = P_h @ V_h
    O = (U @ p_w.T);  O /= ||O||_2(row)
    out = concat([E, O], -1)   [256, 2048]

Sharding: pure data-parallel over batch B=8 across the 8 NeuronCores (one
batch per core, no collectives). Host pre-transposes/casts activations and
weights to bf16 so every matmul has its contraction dim on SBUF partitions,
and re-assembles the output (E-passthrough concat happens on host).

Per-core dataflow (all matmuls bf16 with f32 PSUM accumulation):
  Q^T [1024,256] and K^T [1024,4096] in o-on-partitions layout, so partition
  tile j holds head pair (2j, 2j+1) in rows 0:64 / 64:128 -> QK^T runs as
  concurrent row-group-tiled matmuls producing S^T [m2, n1]. exp() on ScalarE
  (PSUM->SBUF, 1024-wide ops). V in natural [m2, o] layout with a ones column
  appended per head (65-wide stationary) so AV yields U^T rows 0:64 plus the
  softmax row-sum in row 64. Division by the row-sum uses a ones-matmul
  partition broadcast. P-projection consumes U^T directly; the final L2 norm
  reduces over partitions with a ones-matmul and applies 1/sqrt via
  exp(-0.5*ln(x)) on ScalarE.
"""

import os

import numpy as np
import ml_dtypes

B, N1, N2, D = 8, 256, 4096, 1024
H, HD = 16, 64
PAIRS = H // 2  # 8 partition-tiles of head pairs
N_CORES = 8

BF16 = ml_dtypes.bfloat16

_COMPILED = None  # (nc,) cache so repeated kernel() calls skip the rebuild
LAST_RESULT = None  # BassKernelResults of the most recent run (for harnesses)


def _build():
    import concourse.bacc as bacc
    import concourse.mybir as mybir
    from concourse import tile
    from contextlib import ExitStack

    _ph = os.environ.get("KERNEL_PHASE", "5")
    phase = int(_ph[0])
    sub = _ph[1:]

    dt = mybir.dt
    Exp = mybir.ActivationFunctionType.Exp
    Ln = mybir.ActivationFunctionType.Ln
    DR = mybir.MatmulPerfMode.DoubleRow

    nc = bacc.Bacc("TRN2", target_bir_lowering=False, debug=False,
                   num_devices=N_CORES)

    # ---- per-core DRAM tensors (host pre-transposed / pre-cast) ----
    # Activations and weights are fp8(e4m3); weights carry a x64 scale so
    # their mass sits in fp8's normal range. The scale cancels downstream:
    # exp() applies 2^-12 to Q.K, the softmax divide is scale-invariant
    # (ones column = 64 matches V's x64), and the final L2 norm kills the
    # P-projection's x64. Biases are pre-scaled on host to match.
    it_d = nc.dram_tensor("it", [D, N2], dt.float8e4, kind="ExternalInput")
    et_d = nc.dram_tensor("et", [D, N1], dt.float8e4, kind="ExternalInput")
    qw_d = nc.dram_tensor("qw", [D, D], dt.float8e4, kind="ExternalInput")
    kw_d = nc.dram_tensor("kw", [D, D], dt.float8e4, kind="ExternalInput")
    vw_d = nc.dram_tensor("vw", [D, D], dt.float8e4, kind="ExternalInput")
    pw_d = nc.dram_tensor("pw", [D, D], dt.float8e4, kind="ExternalInput")
    qb_d = nc.dram_tensor("qb", [128, 8], dt.float32, kind="ExternalInput")
    kb_d = nc.dram_tensor("kb", [128, 8], dt.float32, kind="ExternalInput")
    pb_d = nc.dram_tensor("pb", [128, 8], dt.float32, kind="ExternalInput")
    vb_d = nc.dram_tensor("vb", [64, 16], dt.float32, kind="ExternalInput")
    ot_d = nc.dram_tensor("ot", [D, N1], dt.float32, kind="ExternalOutput")

    with tile.TileContext(nc) as tc, ExitStack() as top:
        # ---- long-lived SBUF tiles ----
        persist = top.enter_context(tc.tile_pool(name="persist", bufs=1))
        qt_sb = persist.tile([128, PAIRS, N1], dt.bfloat16, tag="qt")   # Q^T
        # odd-head halves relocated to partitions 0:64 (operands at SBUF base
        # partition 64 fault the PE on this hardware — re-confirmed)
        qt_o = persist.tile([64, PAIRS, N1], dt.bfloat16, tag="qt_o")
        v_sb = persist.tile([128, 512, 65], dt.float8e4, tag="v")       # [V|1]
        ut_sb = persist.tile([128, PAIRS, N1], dt.float8e4, tag="ut")   # U^T
        ot_sb = persist.tile([128, 8, N1], dt.float32, tag="ot")        # O^T
        pw_sb = persist.tile([128, 8, D], dt.float8e4, tag="pw")
        qb_sb = persist.tile([128, 8], dt.float32, tag="qb")
        kb_sb = persist.tile([128, 8], dt.float32, tag="kb")
        pb_sb = persist.tile([128, 8], dt.float32, tag="pb")
        vb_sb = persist.tile([64, 16], dt.float32, tag="vb")
        ones_bf = persist.tile([128, 1], dt.bfloat16, tag="ones_bf")
        ones_f0 = persist.tile([1, 128], dt.float32, tag="ones_f0")
        nbias = persist.tile([128, 1], dt.float32, tag="nbias")
        ones_b0 = persist.tile([1, 128], dt.bfloat16, tag="ones_b0")
        nc.vector.memset(nbias[:], -9.0)
        nc.vector.memset(ones_b0[:], 1.0)

        nc.sync.dma_start(qb_sb[:], qb_d[:])
        nc.sync.dma_start(kb_sb[:], kb_d[:])
        nc.sync.dma_start(pb_sb[:], pb_d[:])
        nc.sync.dma_start(vb_sb[:], vb_d[:])
        if phase < 5:
            nc.vector.memset(ut_sb[:], 0.0)
            nc.vector.memset(ot_sb[:], 0.0)
        nc.vector.memset(ones_bf[:], 1.0)
        nc.vector.memset(ones_f0[:], 1.0)
        # ones column of [V|1]: softmax row-sum lands on PSUM partition 64.
        # V itself carries the weights' x64 scale, so the column is 64 to
        # keep the AV-numerator/denominator ratio at true scale.
        nc.vector.memset(v_sb[:, :, 64:65], 64.0)

        with ExitStack() as proj:
            wpool = proj.enter_context(tc.tile_pool(name="wpool", bufs=1))
            itp = proj.enter_context(tc.tile_pool(name="itp", bufs=1))
            pps = proj.enter_context(
                tc.tile_pool(name="pps", bufs=2, space="PSUM"))

            kw_sb = wpool.tile([128, 8, D], dt.float8e4, tag="kw")
            it_sb = itp.tile([128, 8, N2], dt.float8e4, tag="it")

            def emit_it_chunk(ch):
                for c in range(8):
                    nc.sync.dma_start(
                        it_sb[:, c, ch * 1024:(ch + 1) * 1024],
                        it_d[c * 128:(c + 1) * 128, ch * 1024:(ch + 1) * 1024])

            with ExitStack() as qscope:
                qep = qscope.enter_context(tc.tile_pool(name="qep", bufs=1))
                et_sb = qep.tile([128, 8, N1], dt.float8e4, tag="et")
                qw_sb = qep.tile([128, 8, D], dt.float8e4, tag="qw")
                for c in range(8):
                    nc.sync.dma_start(et_sb[:, c, :],
                                      et_d[c * 128:(c + 1) * 128, :])
                # j-major so the j=0 matmuls start after ~1/8 of the bytes
                for j in range(PAIRS):
                    for c in range(8):
                        nc.sync.dma_start(
                            qw_sb[:, c, j * 128:(j + 1) * 128],
                            qw_d[c * 128:(c + 1) * 128,
                                 j * 128:(j + 1) * 128])
                emit_it_chunk(0)
                # ---- Q^T projection (fp8 DoubleRow: c-pairs of k-subtiles) ----
                for j in range(PAIRS):
                    ps = pps.tile([128, 512], dt.float32, tag="qk_ps")
                    for cp in range(4):
                        nc.tensor.matmul(
                            ps[:, 0:N1],
                            qw_sb[:, 2 * cp:2 * cp + 2, j * 128:(j + 1) * 128],
                            et_sb[:, 2 * cp:2 * cp + 2, :],
                            start=(cp == 0), stop=(cp == 3), perf_mode=DR)
                    nc.vector.tensor_scalar_add(qt_sb[:, j, :], ps[:, 0:N1],
                                                qb_sb[:, j:j + 1])
                    nc.sync.dma_start(qt_o[:, j, :], qt_sb[64:128, j, :])

            # ---- V projection (natural layout, strided into [V|1] slots) ----
            with ExitStack() as vscope:
                vwp = vscope.enter_context(tc.tile_pool(name="vwp", bufs=1))
                vw_sb = vwp.tile([128, 8, D], dt.float8e4, tag="vw")
                for c in range(8):
                    nc.sync.dma_start(vw_sb[:, c, :],
                                      vw_d[c * 128:(c + 1) * 128, :])
                emit_it_chunk(1)
                for c in range(8):
                    nc.sync.dma_start(kw_sb[:, c, :],
                                      kw_d[c * 128:(c + 1) * 128, :])
                for ch in range(2, 4):
                    emit_it_chunk(ch)
                for c in range(8):
                    nc.sync.dma_start(pw_sb[:, c, :],
                                      pw_d[c * 128:(c + 1) * 128, :])
                vps = vscope.enter_context(
                    tc.tile_pool(name="vps", bufs=3, space="PSUM"))
                for t in range(32):
                    # the it-chunk stationary is reused by both output halves
                    ps0 = vps.tile([128, 512], dt.float32, tag="v_ps")
                    ps1 = vps.tile([128, 512], dt.float32, tag="v_ps")
                    pss = (ps0, ps1)
                    for cp in range(4):
                        for s in range(2):
                            nc.tensor.matmul(
                                pss[s][:],
                                it_sb[:, 2 * cp:2 * cp + 2,
                                      t * 128:(t + 1) * 128],
                                vw_sb[:, 2 * cp:2 * cp + 2,
                                      s * 512:(s + 1) * 512],
                                start=(cp == 0), stop=(cp == 3), perf_mode=DR)
                    for s in range(2):
                        dst = v_sb[:, t * 16 + s * 8: t * 16 + s * 8 + 8, 0:64]
                        nc.vector.tensor_copy(dst, pss[s][:].rearrange(
                            "p (h d) -> p h d", d=64))

            # ---- K^T projection interleaved with attention ----
            ktp = proj.enter_context(tc.tile_pool(name="ktp", bufs=2))
            ktop = proj.enter_context(tc.tile_pool(name="ktop", bufs=2))
            sps = proj.enter_context(
                tc.tile_pool(name="sps", bufs=2, space="PSUM"))
            avp = proj.enter_context(
                tc.tile_pool(name="avp", bufs=2, space="PSUM"))
            ptp = proj.enter_context(tc.tile_pool(name="ptp", bufs=3))
            invp = proj.enter_context(tc.tile_pool(name="invp", bufs=1))
            bsbp = proj.enter_context(tc.tile_pool(name="bsbp", bufs=1))
            stp = proj.enter_context(tc.tile_pool(name="stp", bufs=1))

            kt_tiles = []

            def emit_k_block(j, g8, kt, kt_o):
                ps = pps.tile([128, 512], dt.float32, tag="qk_ps")
                for cp in range(4):
                    nc.tensor.matmul(
                        ps[:],
                        kw_sb[:, 2 * cp:2 * cp + 2, j * 128:(j + 1) * 128],
                        it_sb[:, 2 * cp:2 * cp + 2,
                              g8 * 512:(g8 + 1) * 512],
                        start=(cp == 0), stop=(cp == 3), perf_mode=DR)
                nc.vector.tensor_scalar_add(
                    kt[:, g8 * 512:(g8 + 1) * 512], ps[:], kb_sb[:, j:j + 1])
                nc.sync.dma_start(kt_o[:, g8 * 512:(g8 + 1) * 512],
                                  kt[64:128, g8 * 512:(g8 + 1) * 512])

            def emit_qk_exp(p, kt, kt_o, g):
                """QK^T + exp for group g (m2 tiles 2g, 2g+1); returns pt."""
                s_ps = sps.tile([128, 1024], dt.float32, tag="s_ps")
                for u in range(2):
                    t = 2 * g + u
                    nc.tensor.matmul(
                        s_ps[:, u * 512: u * 512 + 256],
                        kt[0:64, t * 128:(t + 1) * 128],
                        qt_sb[0:64, p, :], start=True, stop=True)
                    nc.tensor.matmul(
                        s_ps[:, u * 512 + 256: u * 512 + 512],
                        kt_o[:, t * 128:(t + 1) * 128],
                        qt_o[:, p, :], start=True, stop=True)
                pt = ptp.tile([128, 1024], dt.float8e5, tag="pt")
                # Q and K both carry x64 -> S is 4096x; exp's scale undoes it
                # and bias -9 recentres P into fp8e5's range (cancels in the
                # softmax numerator/denominator ratio)
                nc.scalar.activation(pt[:], s_ps[:], Exp, scale=2.0 ** -12,
                                     bias=nbias[:])
                return pt

            def emit_av(p, av_ab, g, pt):
                """fp8 DoubleRow AV: each matmul covers both m2 tiles of g."""
                ptu = pt[:].rearrange("q (u x) -> q u x", u=2)
                vr = v_sb[:].rearrange("q (g u h) d -> q g u h d", u=2, h=16)
                for a in range(2):
                    h = 2 * p + a
                    nc.tensor.matmul(
                        av_ab[a][0:65, :],
                        vr[:, g, :, h, :],
                        ptu[:, :, a * 256:(a + 1) * 256],
                        start=(g == 0), stop=(g == 15), perf_mode=DR)

            AV_DELAY = 2  # groups of lag so exp() hides under later QK work

            def emit_pair_finalize(p, av_ab):
                if phase == 2:
                    if sub == "a":
                        return
                    # dump accumulators without the broadcast-divide machinery
                    nc.vector.tensor_copy(ut_sb[0:64, p, :], av_ab[0][0:64, :])
                    st2 = stp.tile([64, N1], dt.bfloat16, tag="st")
                    nc.vector.tensor_copy(st2[:], av_ab[1][0:64, :])
                    nc.sync.dma_start(ut_sb[64:128, p, :], st2[:])
                    return
                # evict undivided U^T halves + denominators first so the AV
                # PSUM banks free before the broadcast-divide chain runs
                ue = bsbp.tile([64, 512], dt.bfloat16, tag="ue")
                nc.vector.tensor_copy(ue[:, 0:256], av_ab[0][0:64, :])
                nc.vector.tensor_copy(ue[:, 256:512], av_ab[1][0:64, :])
                # row 64 of each AV accumulator is the softmax denominator;
                # reciprocal on partition 64, then shift the row to partition 0
                inv = invp.tile([65, 512], dt.bfloat16, tag="inv")
                with nc.allow_low_precision("softmax denom; ~0.4% rel"):
                    nc.vector.reciprocal(inv[64:65, 0:256],
                                         av_ab[0][64:65, :])
                    nc.vector.reciprocal(inv[64:65, 256:512],
                                         av_ab[1][64:65, :])
                nc.sync.dma_start(inv[0:1, :], inv[64:65, :])
                bc_ps = pps.tile([128, 512], dt.float32, tag="qk_ps")
                nc.tensor.matmul(bc_ps[:], ones_b0[:], inv[0:1, :],
                                 start=True, stop=True)
                bc_sb = bsbp.tile([64, 512], dt.float32, tag="bc_sb")
                nc.vector.tensor_copy(bc_sb[:], bc_ps[0:64, :])
                # even head: divide + v_b straight into rows 0:64 of U^T
                nc.vector.tensor_mul(ut_sb[0:64, p, :], ue[:, 0:256],
                                     bc_sb[:, 0:256])
                nc.vector.tensor_scalar_add(
                    ut_sb[0:64, p, :], ut_sb[0:64, p, :],
                    vb_sb[:, 2 * p:2 * p + 1])
                # odd head: staging, then partition-shift DMA to rows 64:128
                st = stp.tile([64, N1], dt.float8e4, tag="st")
                nc.vector.tensor_mul(st[:], ue[:, 256:512],
                                     bc_sb[:, 256:512])
                nc.vector.tensor_scalar_add(st[:], st[:],
                                            vb_sb[:, 2 * p + 1:2 * p + 2])
                nc.sync.dma_start(ut_sb[64:128, p, :], st[:])

            def emit_pair_attn(p, av_ab, kt, kt_o, chunk):
                """Attention for pair p, AV lagging QK/exp by AV_DELAY groups.

                chunk: None = all 16 groups in one go; else g8 index whose
                two groups to emit (interleaved with K-proj of the next pair).
                """
                pend = pend_by_pair.setdefault(p, [])
                groups = range(16) if chunk is None else (2 * chunk,
                                                          2 * chunk + 1)
                for g in groups:
                    pend.append((g, emit_qk_exp(p, kt, kt_o, g)))
                    if len(pend) > AV_DELAY:
                        emit_av(p, av_ab, *pend.pop(0))
                if (chunk is None or chunk == 7) and phase >= 2:
                    for item in pend:
                        emit_av(p, av_ab, *item)
                    pend.clear()
                    emit_pair_finalize(p, av_ab)

            pend_by_pair = {}
            prev = None  # (pair_idx, (av_a, av_b), kt, kt_o)
            for j in range(PAIRS):
                kt = ktp.tile([128, N2], dt.bfloat16, tag="kt")
                kt_o = ktop.tile([64, N2], dt.bfloat16, tag="kt_o")
                kt_tiles.append(kt)
                for g8 in range(8):
                    emit_k_block(j, g8, kt, kt_o)
                    if prev is not None and phase >= 2:
                        emit_pair_attn(prev[0], prev[1], prev[2], prev[3], g8)
                av = avp.tile([128, 2 * N1], dt.float32, tag="av")
                av_ab = (av[:, 0:N1], av[:, N1:2 * N1])
                prev = (j, av_ab, kt, kt_o)
            if phase >= 2:
                emit_pair_attn(prev[0], prev[1], prev[2], prev[3], None)
            if phase == 1:
                # keep K^T tiles alive / observable: dump slices into O^T
                for co in range(8):
                    nc.vector.tensor_copy(ot_sb[:, co, 0:N1],
                                          kt_tiles[co][:, 0:N1])

        # ---- P projection + L2 normalize (projection pools freed) ----
        with ExitStack() as tail:
            ops = tail.enter_context(
                tc.tile_pool(name="ops", bufs=2, space="PSUM"))
            nps = tail.enter_context(
                tc.tile_pool(name="nps", bufs=1, space="PSUM"))
            sqp = tail.enter_context(tc.tile_pool(name="sqp", bufs=2))
            fop = tail.enter_context(tc.tile_pool(name="fop", bufs=2))

            if phase >= 4:
                for co in range(8):
                    ps = ops.tile([128, N1], dt.float32, tag="o_ps")
                    for cp in range(4):
                        nc.tensor.matmul(
                            ps[:],
                            pw_sb[:, 2 * cp:2 * cp + 2,
                                  co * 128:(co + 1) * 128],
                            ut_sb[:, 2 * cp:2 * cp + 2, :],
                            start=(cp == 0), stop=(cp == 3), perf_mode=DR)
                    nc.vector.tensor_scalar_add(ot_sb[:, co, :], ps[:],
                                                pb_sb[:, co:co + 1])

            if phase >= 5:
                nsq = nps.tile([128, N1], dt.float32, tag="nsq")
                for co in range(8):
                    sq = sqp.tile([128, N1], dt.bfloat16, tag="sq")
                    nc.vector.tensor_mul(sq[:], ot_sb[:, co, :],
                                         ot_sb[:, co, :])
                    nc.tensor.matmul(nsq[0:1, :], ones_bf[:], sq[:],
                                     start=(co == 0), stop=(co == 7))
                lnt = sqp.tile([1, N1], dt.float32, tag="lnt")
                nc.scalar.activation(lnt[:], nsq[0:1, :], Ln)
                invn = sqp.tile([1, N1], dt.bfloat16, tag="invn")
                nc.scalar.activation(invn[:], lnt[:], Exp, scale=-0.5)
                bcn = nps.tile([128, N1], dt.float32, tag="bcn")
                nc.tensor.matmul(bcn[:], ones_b0[:], invn[:],
                                 start=True, stop=True)
                for co in range(8):
                    fo = fop.tile([128, N1], dt.float32, tag="fo")
                    nc.vector.tensor_mul(fo[:], ot_sb[:, co, :], bcn[:])
                    nc.sync.dma_start(ot_d[co * 128:(co + 1) * 128, :], fo[:])
            else:
                for co in range(8):
                    fo = fop.tile([128, N1], dt.float32, tag="fo")
                    nc.vector.tensor_copy(fo[:], ot_sb[:, co, :])
                    nc.sync.dma_start(ot_d[co * 128:(co + 1) * 128, :], fo[:])

    nc.compile()
    return nc


def kernel(E, I, q_w, q_b, k_w, k_b, v_w, v_b, p_w, p_b):
    global _COMPILED, LAST_RESULT
    from concourse import bass_utils

    if _COMPILED is None:
        _COMPILED = _build()
    nc = _COMPILED

    E = np.asarray(E, dtype=np.float32)
    I = np.asarray(I, dtype=np.float32)
    F8 = ml_dtypes.float8_e4m3

    def _wT(w):
        # x64 lifts the (0.02-scale) weights into fp8's normal range; the
        # scale cancels on-device (see _build)
        return np.ascontiguousarray(np.asarray(w, np.float32).T * 64.0
                                    ).astype(F8)

    qw, kw, vw, pw = _wT(q_w), _wT(k_w), _wT(v_w), _wT(p_w)
    qb = np.ascontiguousarray(
        np.asarray(q_b, np.float32).reshape(8, 128).T * 64.0)
    kb = np.ascontiguousarray(
        np.asarray(k_b, np.float32).reshape(8, 128).T * 64.0)
    pb = np.ascontiguousarray(
        np.asarray(p_b, np.float32).reshape(8, 128).T * 64.0)
    vb = np.ascontiguousarray(np.asarray(v_b, np.float32).reshape(16, 64).T)

    in_maps = []
    for b in range(B):
        in_maps.append({
            "it": np.ascontiguousarray(I[b].T).astype(F8),
            "et": np.ascontiguousarray(E[b].T).astype(F8),
            "qw": qw, "kw": kw, "vw": vw, "pw": pw,
            "qb": qb, "kb": kb, "pb": pb, "vb": vb,
        })

    res = bass_utils.run_bass_kernel_spmd(
        nc, in_maps, core_ids=list(range(N_CORES)),
        trace=bool(os.environ.get("BASS_TRACE")))
    LAST_RESULT = res

    out = np.empty((B, N1, 2048), dtype=np.float32)
    for b in range(B):
        out[b, :, :1024] = E[b]
        out[b, :, 1024:] = res.results[b]["ot"].T
    return out



# revision 42
# speedup vs baseline: 1.0979x; 1.0979x over previous
"""Trainium2 Bass kernel for nn_ContextEmbedding (cross-attention context embedding).

Reference math (per batch b):
    Q = E @ q_w.T        [256, 1024]
    K = I @ k_w.T        [4096, 1024]
    V = I @ v_w.T        [4096, 1024]
    S_h = Q_h @ K_h.T    per head (16 heads, head_dim 64)
    P = softmax(S, -1)
    U_h = P_h @ V_h
    O = (U @ p_w.T);  O /= ||O||_2(row)
    out = concat([E, O], -1)   [256, 2048]

Sharding: pure data-parallel over batch B=8 across the 8 NeuronCores (one
batch per core, no collectives). Host pre-transposes/casts activations and
weights to bf16 so every matmul has its contraction dim on SBUF partitions,
and re-assembles the output (E-passthrough concat happens on host).

Per-core dataflow (all matmuls bf16 with f32 PSUM accumulation):
  Q^T [1024,256] and K^T [1024,4096] in o-on-partitions layout, so partition
  tile j holds head pair (2j, 2j+1) in rows 0:64 / 64:128 -> QK^T runs as
  concurrent row-group-tiled matmuls producing S^T [m2, n1]. exp() on ScalarE
  (PSUM->SBUF, 1024-wide ops). V in natural [m2, o] layout with a ones column
  appended per head (65-wide stationary) so AV yields U^T rows 0:64 plus the
  softmax row-sum in row 64. Division by the row-sum uses a ones-matmul
  partition broadcast. P-projection consumes U^T directly; the final L2 norm
  reduces over partitions with a ones-matmul and applies 1/sqrt via
  exp(-0.5*ln(x)) on ScalarE.
"""

import os

import numpy as np
import ml_dtypes

B, N1, N2, D = 8, 256, 4096, 1024
H, HD = 16, 64
PAIRS = H // 2  # 8 partition-tiles of head pairs
N_CORES = 8

BF16 = ml_dtypes.bfloat16

_COMPILED = None  # (nc,) cache so repeated kernel() calls skip the rebuild
LAST_RESULT = None  # BassKernelResults of the most recent run (for harnesses)


def _build():
    import concourse.bacc as bacc
    import concourse.mybir as mybir
    from concourse import tile
    from contextlib import ExitStack

    _ph = os.environ.get("KERNEL_PHASE", "5")
    phase = int(_ph[0])
    sub = _ph[1:]

    dt = mybir.dt
    Exp = mybir.ActivationFunctionType.Exp
    Ln = mybir.ActivationFunctionType.Ln
    DR = mybir.MatmulPerfMode.DoubleRow

    nc = bacc.Bacc("TRN2", target_bir_lowering=False, debug=False,
                   num_devices=N_CORES)

    # ---- per-core DRAM tensors (host pre-transposed / pre-cast) ----
    # Activations and weights are fp8(e4m3); weights carry a x64 scale so
    # their mass sits in fp8's normal range. The scale cancels downstream:
    # exp() applies 2^-12 to Q.K, the softmax divide is scale-invariant
    # (ones column = 64 matches V's x64), and the final L2 norm kills the
    # P-projection's x64. Biases are pre-scaled on host to match.
    it_d = nc.dram_tensor("it", [D, N2], dt.float8e4, kind="ExternalInput")
    et_d = nc.dram_tensor("et", [D, N1], dt.float8e4, kind="ExternalInput")
    qw_d = nc.dram_tensor("qw", [D, D], dt.float8e4, kind="ExternalInput")
    kw_d = nc.dram_tensor("kw", [D, D], dt.float8e4, kind="ExternalInput")
    vw_d = nc.dram_tensor("vw", [D, D], dt.float8e4, kind="ExternalInput")
    pw_d = nc.dram_tensor("pw", [D, D], dt.float8e4, kind="ExternalInput")
    qb_d = nc.dram_tensor("qb", [128, 8], dt.float32, kind="ExternalInput")
    kb_d = nc.dram_tensor("kb", [128, 8], dt.float32, kind="ExternalInput")
    pb_d = nc.dram_tensor("pb", [128, 8], dt.float32, kind="ExternalInput")
    vb_d = nc.dram_tensor("vb", [64, 16], dt.float32, kind="ExternalInput")
    ot_d = nc.dram_tensor("ot", [D, N1], dt.float32, kind="ExternalOutput")

    with tile.TileContext(nc) as tc, ExitStack() as top:
        # ---- long-lived SBUF tiles ----
        persist = top.enter_context(tc.tile_pool(name="persist", bufs=1))
        qt_sb = persist.tile([128, PAIRS, N1], dt.bfloat16, tag="qt")   # Q^T
        # odd-head halves relocated to partitions 0:64 (operands at SBUF base
        # partition 64 fault the PE on this hardware — re-confirmed)
        qt_o = persist.tile([64, PAIRS, N1], dt.bfloat16, tag="qt_o")
        v_sb = persist.tile([128, 512, 65], dt.float8e4, tag="v")       # [V|1]
        ut_sb = persist.tile([128, PAIRS, N1], dt.float8e4, tag="ut")   # U^T
        ot_sb = persist.tile([128, 8, N1], dt.float32, tag="ot")        # O^T
        pw_sb = persist.tile([128, 8, D], dt.float8e4, tag="pw")
        qb_sb = persist.tile([128, 8], dt.float32, tag="qb")
        kb_sb = persist.tile([128, 8], dt.float32, tag="kb")
        pb_sb = persist.tile([128, 8], dt.float32, tag="pb")
        vb_sb = persist.tile([64, 16], dt.float32, tag="vb")
        ones_bf = persist.tile([128, 1], dt.bfloat16, tag="ones_bf")
        ones_f0 = persist.tile([1, 128], dt.float32, tag="ones_f0")
        nbias = persist.tile([128, 1], dt.float32, tag="nbias")
        ones_b0 = persist.tile([1, 128], dt.bfloat16, tag="ones_b0")
        nc.vector.memset(nbias[:], -9.0)
        nc.vector.memset(ones_b0[:], 1.0)

        nc.sync.dma_start(qb_sb[:], qb_d[:])
        nc.sync.dma_start(kb_sb[:], kb_d[:])
        nc.sync.dma_start(pb_sb[:], pb_d[:])
        nc.sync.dma_start(vb_sb[:], vb_d[:])
        if phase < 5:
            nc.vector.memset(ut_sb[:], 0.0)
            nc.vector.memset(ot_sb[:], 0.0)
        nc.vector.memset(ones_bf[:], 1.0)
        nc.vector.memset(ones_f0[:], 1.0)
        # ones column of [V|1]: softmax row-sum lands on PSUM partition 64.
        # V itself carries the weights' x64 scale, so the column is 64 to
        # keep the AV-numerator/denominator ratio at true scale.
        nc.vector.memset(v_sb[:, :, 64:65], 64.0)

        with ExitStack() as proj:
            wpool = proj.enter_context(tc.tile_pool(name="wpool", bufs=1))
            itp = proj.enter_context(tc.tile_pool(name="itp", bufs=1))
            pps = proj.enter_context(
                tc.tile_pool(name="pps", bufs=2, space="PSUM"))

            kw_sb = wpool.tile([128, 8, D], dt.float8e4, tag="kw")
            it_sb = itp.tile([128, 8, N2], dt.float8e4, tag="it")

            def emit_it_chunk(ch):
                for c in range(8):
                    nc.sync.dma_start(
                        it_sb[:, c, ch * 1024:(ch + 1) * 1024],
                        it_d[c * 128:(c + 1) * 128, ch * 1024:(ch + 1) * 1024])

            with ExitStack() as qscope:
                qep = qscope.enter_context(tc.tile_pool(name="qep", bufs=1))
                et_sb = qep.tile([128, 8, N1], dt.float8e4, tag="et")
                qw_sb = qep.tile([128, 8, D], dt.float8e4, tag="qw")
                for c in range(8):
                    nc.sync.dma_start(et_sb[:, c, :],
                                      et_d[c * 128:(c + 1) * 128, :])
                    nc.sync.dma_start(qw_sb[:, c, :],
                                      qw_d[c * 128:(c + 1) * 128, :])
                emit_it_chunk(0)
                # ---- Q^T projection (fp8 DoubleRow: c-pairs of k-subtiles) ----
                for j in range(PAIRS):
                    ps = pps.tile([128, 512], dt.float32, tag="qk_ps")
                    for cp in range(4):
                        nc.tensor.matmul(
                            ps[:, 0:N1],
                            qw_sb[:, 2 * cp:2 * cp + 2, j * 128:(j + 1) * 128],
                            et_sb[:, 2 * cp:2 * cp + 2, :],
                            start=(cp == 0), stop=(cp == 3), perf_mode=DR)
                    nc.vector.tensor_scalar_add(qt_sb[:, j, :], ps[:, 0:N1],
                                                qb_sb[:, j:j + 1])
                    nc.sync.dma_start(qt_o[:, j, :], qt_sb[64:128, j, :])

            # ---- V projection (natural layout, strided into [V|1] slots) ----
            with ExitStack() as vscope:
                vwp = vscope.enter_context(tc.tile_pool(name="vwp", bufs=1))
                vw_sb = vwp.tile([128, 8, D], dt.float8e4, tag="vw")
                for c in range(8):
                    nc.sync.dma_start(vw_sb[:, c, :],
                                      vw_d[c * 128:(c + 1) * 128, :])
                for ch in range(1, 4):
                    emit_it_chunk(ch)
                # kw/pw after the it chunks: they are needed later and must
                # not delay the V-projection's input stream
                for c in range(8):
                    nc.sync.dma_start(kw_sb[:, c, :],
                                      kw_d[c * 128:(c + 1) * 128, :])
                for c in range(8):
                    nc.sync.dma_start(pw_sb[:, c, :],
                                      pw_d[c * 128:(c + 1) * 128, :])
                vps = vscope.enter_context(
                    tc.tile_pool(name="vps", bufs=3, space="PSUM"))
                for t in range(32):
                    # the it-chunk stationary is reused by both output halves
                    ps0 = vps.tile([128, 512], dt.float32, tag="v_ps")
                    ps1 = vps.tile([128, 512], dt.float32, tag="v_ps")
                    pss = (ps0, ps1)
                    for cp in range(4):
                        for s in range(2):
                            nc.tensor.matmul(
                                pss[s][:],
                                it_sb[:, 2 * cp:2 * cp + 2,
                                      t * 128:(t + 1) * 128],
                                vw_sb[:, 2 * cp:2 * cp + 2,
                                      s * 512:(s + 1) * 512],
                                start=(cp == 0), stop=(cp == 3), perf_mode=DR)
                    for s in range(2):
                        dst = v_sb[:, t * 16 + s * 8: t * 16 + s * 8 + 8, 0:64]
                        nc.vector.tensor_copy(dst, pss[s][:].rearrange(
                            "p (h d) -> p h d", d=64))

            # ---- K^T projection interleaved with attention ----
            ktp = proj.enter_context(tc.tile_pool(name="ktp", bufs=2))
            ktop = proj.enter_context(tc.tile_pool(name="ktop", bufs=2))
            sps = proj.enter_context(
                tc.tile_pool(name="sps", bufs=2, space="PSUM"))
            avp = proj.enter_context(
                tc.tile_pool(name="avp", bufs=2, space="PSUM"))
            ptp = proj.enter_context(tc.tile_pool(name="ptp", bufs=3))
            invp = proj.enter_context(tc.tile_pool(name="invp", bufs=1))
            bsbp = proj.enter_context(tc.tile_pool(name="bsbp", bufs=1))
            stp = proj.enter_context(tc.tile_pool(name="stp", bufs=1))

            kt_tiles = []

            def emit_k_block(j, g8, kt, kt_o):
                ps = pps.tile([128, 512], dt.float32, tag="qk_ps")
                for cp in range(4):
                    nc.tensor.matmul(
                        ps[:],
                        kw_sb[:, 2 * cp:2 * cp + 2, j * 128:(j + 1) * 128],
                        it_sb[:, 2 * cp:2 * cp + 2,
                              g8 * 512:(g8 + 1) * 512],
                        start=(cp == 0), stop=(cp == 3), perf_mode=DR)
                nc.vector.tensor_scalar_add(
                    kt[:, g8 * 512:(g8 + 1) * 512], ps[:], kb_sb[:, j:j + 1])
                nc.sync.dma_start(kt_o[:, g8 * 512:(g8 + 1) * 512],
                                  kt[64:128, g8 * 512:(g8 + 1) * 512])

            def emit_qk_exp(p, kt, kt_o, g):
                """QK^T + exp for group g (m2 tiles 2g, 2g+1); returns pt."""
                s_ps = sps.tile([128, 1024], dt.float32, tag="s_ps")
                for u in range(2):
                    t = 2 * g + u
                    nc.tensor.matmul(
                        s_ps[:, u * 512: u * 512 + 256],
                        kt[0:64, t * 128:(t + 1) * 128],
                        qt_sb[0:64, p, :], start=True, stop=True)
                    nc.tensor.matmul(
                        s_ps[:, u * 512 + 256: u * 512 + 512],
                        kt_o[:, t * 128:(t + 1) * 128],
                        qt_o[:, p, :], start=True, stop=True)
                pt = ptp.tile([128, 1024], dt.float8e5, tag="pt")
                # Q and K both carry x64 -> S is 4096x; exp's scale undoes it
                # and bias -9 recentres P into fp8e5's range (cancels in the
                # softmax numerator/denominator ratio)
                nc.scalar.activation(pt[:], s_ps[:], Exp, scale=2.0 ** -12,
                                     bias=nbias[:])
                return pt

            def emit_av(p, av_ab, g, pt):
                """fp8 DoubleRow AV: each matmul covers both m2 tiles of g."""
                ptu = pt[:].rearrange("q (u x) -> q u x", u=2)
                vr = v_sb[:].rearrange("q (g u h) d -> q g u h d", u=2, h=16)
                for a in range(2):
                    h = 2 * p + a
                    nc.tensor.matmul(
                        av_ab[a][0:65, :],
                        vr[:, g, :, h, :],
                        ptu[:, :, a * 256:(a + 1) * 256],
                        start=(g == 0), stop=(g == 15), perf_mode=DR)

            AV_DELAY = 2  # groups of lag so exp() hides under later QK work

            def emit_pair_finalize(p, av_ab):
                if phase == 2:
                    if sub == "a":
                        return
                    # dump accumulators without the broadcast-divide machinery
                    nc.vector.tensor_copy(ut_sb[0:64, p, :], av_ab[0][0:64, :])
                    st2 = stp.tile([64, N1], dt.bfloat16, tag="st")
                    nc.vector.tensor_copy(st2[:], av_ab[1][0:64, :])
                    nc.sync.dma_start(ut_sb[64:128, p, :], st2[:])
                    return
                # evict undivided U^T halves + denominators first so the AV
                # PSUM banks free before the broadcast-divide chain runs
                ue = bsbp.tile([64, 512], dt.bfloat16, tag="ue")
                nc.vector.tensor_copy(ue[:, 0:256], av_ab[0][0:64, :])
                nc.vector.tensor_copy(ue[:, 256:512], av_ab[1][0:64, :])
                # row 64 of each AV accumulator is the softmax denominator;
                # reciprocal on partition 64, then shift the row to partition 0
                inv = invp.tile([65, 512], dt.bfloat16, tag="inv")
                with nc.allow_low_precision("softmax denom; ~0.4% rel"):
                    nc.vector.reciprocal(inv[64:65, 0:256],
                                         av_ab[0][64:65, :])
                    nc.vector.reciprocal(inv[64:65, 256:512],
                                         av_ab[1][64:65, :])
                nc.sync.dma_start(inv[0:1, :], inv[64:65, :])
                bc_ps = pps.tile([128, 512], dt.float32, tag="qk_ps")
                nc.tensor.matmul(bc_ps[:], ones_b0[:], inv[0:1, :],
                                 start=True, stop=True)
                bc_sb = bsbp.tile([64, 512], dt.float32, tag="bc_sb")
                nc.vector.tensor_copy(bc_sb[:], bc_ps[0:64, :])
                # even head: divide + v_b straight into rows 0:64 of U^T
                nc.vector.tensor_mul(ut_sb[0:64, p, :], ue[:, 0:256],
                                     bc_sb[:, 0:256])
                nc.vector.tensor_scalar_add(
                    ut_sb[0:64, p, :], ut_sb[0:64, p, :],
                    vb_sb[:, 2 * p:2 * p + 1])
                # odd head: staging, then partition-shift DMA to rows 64:128
                st = stp.tile([64, N1], dt.float8e4, tag="st")
                nc.vector.tensor_mul(st[:], ue[:, 256:512],
                                     bc_sb[:, 256:512])
                nc.vector.tensor_scalar_add(st[:], st[:],
                                            vb_sb[:, 2 * p + 1:2 * p + 2])
                nc.sync.dma_start(ut_sb[64:128, p, :], st[:])

            def emit_pair_attn(p, av_ab, kt, kt_o, chunk):
                """Attention for pair p, AV lagging QK/exp by AV_DELAY groups.

                chunk: None = all 16 groups in one go; else g8 index whose
                two groups to emit (interleaved with K-proj of the next pair).
                """
                pend = pend_by_pair.setdefault(p, [])
                groups = range(16) if chunk is None else (2 * chunk,
                                                          2 * chunk + 1)
                for g in groups:
                    pend.append((g, emit_qk_exp(p, kt, kt_o, g)))
                    if len(pend) > AV_DELAY:
                        emit_av(p, av_ab, *pend.pop(0))
                if (chunk is None or chunk == 7) and phase >= 2:
                    for item in pend:
                        emit_av(p, av_ab, *item)
                    pend.clear()
                    emit_pair_finalize(p, av_ab)

            pend_by_pair = {}
            prev = None  # (pair_idx, (av_a, av_b), kt, kt_o)
            for j in range(PAIRS):
                kt = ktp.tile([128, N2], dt.bfloat16, tag="kt")
                kt_o = ktop.tile([64, N2], dt.bfloat16, tag="kt_o")
                kt_tiles.append(kt)
                for g8 in range(8):
                    emit_k_block(j, g8, kt, kt_o)
                    if prev is not None and phase >= 2:
                        emit_pair_attn(prev[0], prev[1], prev[2], prev[3], g8)
                av = avp.tile([128, 2 * N1], dt.float32, tag="av")
                av_ab = (av[:, 0:N1], av[:, N1:2 * N1])
                prev = (j, av_ab, kt, kt_o)
            if phase >= 2:
                emit_pair_attn(prev[0], prev[1], prev[2], prev[3], None)
            if phase == 1:
                # keep K^T tiles alive / observable: dump slices into O^T
                for co in range(8):
                    nc.vector.tensor_copy(ot_sb[:, co, 0:N1],
                                          kt_tiles[co][:, 0:N1])

        # ---- P projection + L2 normalize (projection pools freed) ----
        with ExitStack() as tail:
            ops = tail.enter_context(
                tc.tile_pool(name="ops", bufs=2, space="PSUM"))
            nps = tail.enter_context(
                tc.tile_pool(name="nps", bufs=1, space="PSUM"))
            sqp = tail.enter_context(tc.tile_pool(name="sqp", bufs=2))
            fop = tail.enter_context(tc.tile_pool(name="fop", bufs=2))

            if phase >= 4:
                for co in range(8):
                    ps = ops.tile([128, N1], dt.float32, tag="o_ps")
                    for cp in range(4):
                        nc.tensor.matmul(
                            ps[:],
                            pw_sb[:, 2 * cp:2 * cp + 2,
                                  co * 128:(co + 1) * 128],
                            ut_sb[:, 2 * cp:2 * cp + 2, :],
                            start=(cp == 0), stop=(cp == 3), perf_mode=DR)
                    nc.vector.tensor_scalar_add(ot_sb[:, co, :], ps[:],
                                                pb_sb[:, co:co + 1])

            if phase >= 5:
                nsq = nps.tile([128, N1], dt.float32, tag="nsq")
                for co in range(8):
                    sq = sqp.tile([128, N1], dt.bfloat16, tag="sq")
                    nc.vector.tensor_mul(sq[:], ot_sb[:, co, :],
                                         ot_sb[:, co, :])
                    nc.tensor.matmul(nsq[0:1, :], ones_bf[:], sq[:],
                                     start=(co == 0), stop=(co == 7))
                lnt = sqp.tile([1, N1], dt.float32, tag="lnt")
                nc.scalar.activation(lnt[:], nsq[0:1, :], Ln)
                invn = sqp.tile([1, N1], dt.bfloat16, tag="invn")
                nc.scalar.activation(invn[:], lnt[:], Exp, scale=-0.5)
                bcn = nps.tile([128, N1], dt.float32, tag="bcn")
                nc.tensor.matmul(bcn[:], ones_b0[:], invn[:],
                                 start=True, stop=True)
                for co in range(8):
                    fo = fop.tile([128, N1], dt.float32, tag="fo")
                    nc.vector.tensor_mul(fo[:], ot_sb[:, co, :], bcn[:])
                    nc.sync.dma_start(ot_d[co * 128:(co + 1) * 128, :], fo[:])
            else:
                for co in range(8):
                    fo = fop.tile([128, N1], dt.float32, tag="fo")
                    nc.vector.tensor_copy(fo[:], ot_sb[:, co, :])
                    nc.sync.dma_start(ot_d[co * 128:(co + 1) * 128, :], fo[:])

    nc.compile()
    return nc


def kernel(E, I, q_w, q_b, k_w, k_b, v_w, v_b, p_w, p_b):
    global _COMPILED, LAST_RESULT
    from concourse import bass_utils

    if _COMPILED is None:
        _COMPILED = _build()
    nc = _COMPILED

    E = np.asarray(E, dtype=np.float32)
    I = np.asarray(I, dtype=np.float32)
    F8 = ml_dtypes.float8_e4m3

    def _wT(w):
        # x64 lifts the (0.02-scale) weights into fp8's normal range; the
        # scale cancels on-device (see _build)
        return np.ascontiguousarray(np.asarray(w, np.float32).T * 64.0
                                    ).astype(F8)

    qw, kw, vw, pw = _wT(q_w), _wT(k_w), _wT(v_w), _wT(p_w)
    qb = np.ascontiguousarray(
        np.asarray(q_b, np.float32).reshape(8, 128).T * 64.0)
    kb = np.ascontiguousarray(
        np.asarray(k_b, np.float32).reshape(8, 128).T * 64.0)
    pb = np.ascontiguousarray(
        np.asarray(p_b, np.float32).reshape(8, 128).T * 64.0)
    vb = np.ascontiguousarray(np.asarray(v_b, np.float32).reshape(16, 64).T)

    in_maps = []
    for b in range(B):
        in_maps.append({
            "it": np.ascontiguousarray(I[b].T).astype(F8),
            "et": np.ascontiguousarray(E[b].T).astype(F8),
            "qw": qw, "kw": kw, "vw": vw, "pw": pw,
            "qb": qb, "kb": kb, "pb": pb, "vb": vb,
        })

    res = bass_utils.run_bass_kernel_spmd(
        nc, in_maps, core_ids=list(range(N_CORES)),
        trace=bool(os.environ.get("BASS_TRACE")))
    LAST_RESULT = res

    out = np.empty((B, N1, 2048), dtype=np.float32)
    for b in range(B):
        out[b, :, :1024] = E[b]
        out[b, :, 1024:] = res.results[b]["ot"].T
    return out



# revision 43
# speedup vs baseline: 1.1132x; 1.0139x over previous
"""Trainium2 Bass kernel for nn_ContextEmbedding (cross-attention context embedding).

Reference math (per batch b):
    Q = E @ q_w.T        [256, 1024]
    K = I @ k_w.T        [4096, 1024]
    V = I @ v_w.T        [4096, 1024]
    S_h = Q_h @ K_h.T    per head (16 heads, head_dim 64)
    P = softmax(S, -1)
    U_h = P_h @ V_h
    O = (U @ p_w.T);  O /= ||O||_2(row)
    out = concat([E, O], -1)   [256, 2048]

Sharding: pure data-parallel over batch B=8 across the 8 NeuronCores (one
batch per core, no collectives). Host pre-transposes/casts activations and
weights to bf16 so every matmul has its contraction dim on SBUF partitions,
and re-assembles the output (E-passthrough concat happens on host).

Per-core dataflow (all matmuls bf16 with f32 PSUM accumulation):
  Q^T [1024,256] and K^T [1024,4096] in o-on-partitions layout, so partition
  tile j holds head pair (2j, 2j+1) in rows 0:64 / 64:128 -> QK^T runs as
  concurrent row-group-tiled matmuls producing S^T [m2, n1]. exp() on ScalarE
  (PSUM->SBUF, 1024-wide ops). V in natural [m2, o] layout with a ones column
  appended per head (65-wide stationary) so AV yields U^T rows 0:64 plus the
  softmax row-sum in row 64. Division by the row-sum uses a ones-matmul
  partition broadcast. P-projection consumes U^T directly; the final L2 norm
  reduces over partitions with a ones-matmul and applies 1/sqrt via
  exp(-0.5*ln(x)) on ScalarE.
"""

import os

import numpy as np
import ml_dtypes

B, N1, N2, D = 8, 256, 4096, 1024
H, HD = 16, 64
PAIRS = H // 2  # 8 partition-tiles of head pairs
N_CORES = 8

BF16 = ml_dtypes.bfloat16

_COMPILED = None  # (nc,) cache so repeated kernel() calls skip the rebuild
LAST_RESULT = None  # BassKernelResults of the most recent run (for harnesses)


def _build():
    import concourse.bacc as bacc
    import concourse.mybir as mybir
    from concourse import tile
    from contextlib import ExitStack

    _ph = os.environ.get("KERNEL_PHASE", "5")
    phase = int(_ph[0])
    sub = _ph[1:]

    dt = mybir.dt
    Exp = mybir.ActivationFunctionType.Exp
    Ln = mybir.ActivationFunctionType.Ln
    DR = mybir.MatmulPerfMode.DoubleRow

    nc = bacc.Bacc("TRN2", target_bir_lowering=False, debug=False,
                   num_devices=N_CORES)

    # ---- per-core DRAM tensors (host pre-transposed / pre-cast) ----
    # Activations and weights are fp8(e4m3); weights carry a x64 scale so
    # their mass sits in fp8's normal range. The scale cancels downstream:
    # exp() applies 2^-12 to Q.K, the softmax divide is scale-invariant
    # (ones column = 64 matches V's x64), and the final L2 norm kills the
    # P-projection's x64. Biases are pre-scaled on host to match.
    it_d = nc.dram_tensor("it", [D, N2], dt.float8e4, kind="ExternalInput")
    et_d = nc.dram_tensor("et", [D, N1], dt.float8e4, kind="ExternalInput")
    qw_d = nc.dram_tensor("qw", [D, D], dt.float8e4, kind="ExternalInput")
    kw_d = nc.dram_tensor("kw", [D, D], dt.float8e4, kind="ExternalInput")
    vw_d = nc.dram_tensor("vw", [D, D], dt.float8e4, kind="ExternalInput")
    pw_d = nc.dram_tensor("pw", [D, D], dt.float8e4, kind="ExternalInput")
    qb_d = nc.dram_tensor("qb", [128, 8], dt.float32, kind="ExternalInput")
    kb_d = nc.dram_tensor("kb", [128, 8], dt.float32, kind="ExternalInput")
    pb_d = nc.dram_tensor("pb", [128, 8], dt.float32, kind="ExternalInput")
    vb_d = nc.dram_tensor("vb", [64, 16], dt.float32, kind="ExternalInput")
    ot_d = nc.dram_tensor("ot", [D, N1], dt.float32, kind="ExternalOutput")

    with tile.TileContext(nc) as tc, ExitStack() as top:
        # ---- long-lived SBUF tiles ----
        persist = top.enter_context(tc.tile_pool(name="persist", bufs=1))
        qt_sb = persist.tile([128, PAIRS, N1], dt.bfloat16, tag="qt")   # Q^T
        # odd-head halves relocated to partitions 0:64 (operands at SBUF base
        # partition 64 fault the PE on this hardware — re-confirmed)
        qt_o = persist.tile([64, PAIRS, N1], dt.bfloat16, tag="qt_o")
        v_sb = persist.tile([128, 512, 65], dt.float8e4, tag="v")       # [V|1]
        ut_sb = persist.tile([128, PAIRS, N1], dt.float8e4, tag="ut")   # U^T
        ot_sb = persist.tile([128, 8, N1], dt.float32, tag="ot")        # O^T
        pw_sb = persist.tile([128, 8, D], dt.float8e4, tag="pw")
        qb_sb = persist.tile([128, 8], dt.float32, tag="qb")
        kb_sb = persist.tile([128, 8], dt.float32, tag="kb")
        pb_sb = persist.tile([128, 8], dt.float32, tag="pb")
        vb_sb = persist.tile([64, 16], dt.float32, tag="vb")
        ones_bf = persist.tile([128, 1], dt.bfloat16, tag="ones_bf")
        ones_f0 = persist.tile([1, 128], dt.float32, tag="ones_f0")
        nbias = persist.tile([128, 1], dt.float32, tag="nbias")
        ones_b0 = persist.tile([1, 128], dt.bfloat16, tag="ones_b0")
        nc.vector.memset(nbias[:], -9.0)
        nc.vector.memset(ones_b0[:], 1.0)

        nc.sync.dma_start(qb_sb[:], qb_d[:])
        nc.sync.dma_start(kb_sb[:], kb_d[:])
        nc.sync.dma_start(pb_sb[:], pb_d[:])
        nc.sync.dma_start(vb_sb[:], vb_d[:])
        if phase < 5:
            nc.vector.memset(ut_sb[:], 0.0)
            nc.vector.memset(ot_sb[:], 0.0)
        nc.vector.memset(ones_bf[:], 1.0)
        nc.vector.memset(ones_f0[:], 1.0)
        # ones column of [V|1]: softmax row-sum lands on PSUM partition 64.
        # V itself carries the weights' x64 scale, so the column is 64 to
        # keep the AV-numerator/denominator ratio at true scale.
        nc.vector.memset(v_sb[:, :, 64:65], 64.0)

        with ExitStack() as proj:
            wpool = proj.enter_context(tc.tile_pool(name="wpool", bufs=1))
            itp = proj.enter_context(tc.tile_pool(name="itp", bufs=1))
            pps = proj.enter_context(
                tc.tile_pool(name="pps", bufs=2, space="PSUM"))

            kw_sb = wpool.tile([128, 8, D], dt.float8e4, tag="kw")
            it_sb = itp.tile([128, 8, N2], dt.float8e4, tag="it")

            def emit_it_chunk(ch):
                for c in range(8):
                    nc.sync.dma_start(
                        it_sb[:, c, ch * 1024:(ch + 1) * 1024],
                        it_d[c * 128:(c + 1) * 128, ch * 1024:(ch + 1) * 1024])

            with ExitStack() as qscope:
                qep = qscope.enter_context(tc.tile_pool(name="qep", bufs=1))
                et_sb = qep.tile([128, 8, N1], dt.float8e4, tag="et")
                qw_sb = qep.tile([128, 8, D], dt.float8e4, tag="qw")
                for c in range(8):
                    nc.sync.dma_start(et_sb[:, c, :],
                                      et_d[c * 128:(c + 1) * 128, :])
                    nc.sync.dma_start(qw_sb[:, c, :],
                                      qw_d[c * 128:(c + 1) * 128, :])
                emit_it_chunk(0)
                # ---- Q^T projection (fp8 DoubleRow: c-pairs of k-subtiles) ----
                for j in range(PAIRS):
                    ps = pps.tile([128, 512], dt.float32, tag="qk_ps")
                    for cp in range(4):
                        nc.tensor.matmul(
                            ps[:, 0:N1],
                            qw_sb[:, 2 * cp:2 * cp + 2, j * 128:(j + 1) * 128],
                            et_sb[:, 2 * cp:2 * cp + 2, :],
                            start=(cp == 0), stop=(cp == 3), perf_mode=DR)
                    nc.vector.tensor_scalar_add(qt_sb[:, j, :], ps[:, 0:N1],
                                                qb_sb[:, j:j + 1])
                    nc.sync.dma_start(qt_o[:, j, :], qt_sb[64:128, j, :])

            # ---- V projection (natural layout, strided into [V|1] slots) ----
            with ExitStack() as vscope:
                vwp = vscope.enter_context(tc.tile_pool(name="vwp", bufs=1))
                vw_sb = vwp.tile([128, 8, D], dt.float8e4, tag="vw")
                for c in range(8):
                    nc.sync.dma_start(vw_sb[:, c, :],
                                      vw_d[c * 128:(c + 1) * 128, :])
                for ch in range(1, 4):
                    emit_it_chunk(ch)
                # kw/pw after the it chunks: they are needed later and must
                # not delay the V-projection's input stream
                for c in range(8):
                    nc.sync.dma_start(kw_sb[:, c, :],
                                      kw_d[c * 128:(c + 1) * 128, :])
                for c in range(8):
                    nc.sync.dma_start(pw_sb[:, c, :],
                                      pw_d[c * 128:(c + 1) * 128, :])
                vps = vscope.enter_context(
                    tc.tile_pool(name="vps", bufs=3, space="PSUM"))
                for t in range(32):
                    # the it-chunk stationary is reused by both output halves
                    ps0 = vps.tile([128, 512], dt.float32, tag="v_ps")
                    ps1 = vps.tile([128, 512], dt.float32, tag="v_ps")
                    pss = (ps0, ps1)
                    for cp in range(4):
                        for s in range(2):
                            nc.tensor.matmul(
                                pss[s][:],
                                it_sb[:, 2 * cp:2 * cp + 2,
                                      t * 128:(t + 1) * 128],
                                vw_sb[:, 2 * cp:2 * cp + 2,
                                      s * 512:(s + 1) * 512],
                                start=(cp == 0), stop=(cp == 3), perf_mode=DR)
                    for s in range(2):
                        dst = v_sb[:, t * 16 + s * 8: t * 16 + s * 8 + 8, 0:64]
                        nc.vector.tensor_copy(dst, pss[s][:].rearrange(
                            "p (h d) -> p h d", d=64))

            # ---- K^T projection interleaved with attention ----
            ktp = proj.enter_context(tc.tile_pool(name="ktp", bufs=2))
            ktop = proj.enter_context(tc.tile_pool(name="ktop", bufs=2))
            sps = proj.enter_context(
                tc.tile_pool(name="sps", bufs=2, space="PSUM"))
            avp = proj.enter_context(
                tc.tile_pool(name="avp", bufs=2, space="PSUM"))
            ptp = proj.enter_context(tc.tile_pool(name="ptp", bufs=3))
            invp = proj.enter_context(tc.tile_pool(name="invp", bufs=1))
            bsbp = proj.enter_context(tc.tile_pool(name="bsbp", bufs=1))
            stp = proj.enter_context(tc.tile_pool(name="stp", bufs=1))

            kt_tiles = []

            def emit_k_block(j, g8, kt, kt_o):
                ps = pps.tile([128, 512], dt.float32, tag="qk_ps")
                for cp in range(4):
                    nc.tensor.matmul(
                        ps[:],
                        kw_sb[:, 2 * cp:2 * cp + 2, j * 128:(j + 1) * 128],
                        it_sb[:, 2 * cp:2 * cp + 2,
                              g8 * 512:(g8 + 1) * 512],
                        start=(cp == 0), stop=(cp == 3), perf_mode=DR)
                nc.vector.tensor_scalar_add(
                    kt[:, g8 * 512:(g8 + 1) * 512], ps[:], kb_sb[:, j:j + 1])
                nc.sync.dma_start(kt_o[:, g8 * 512:(g8 + 1) * 512],
                                  kt[64:128, g8 * 512:(g8 + 1) * 512])

            def emit_qk_exp(p, kt, kt_o, g):
                """QK^T + exp for group g (m2 tiles 2g, 2g+1); returns pt."""
                s_ps = sps.tile([128, 1024], dt.float32, tag="s_ps")
                for u in range(2):
                    t = 2 * g + u
                    nc.tensor.matmul(
                        s_ps[:, u * 512: u * 512 + 256],
                        kt[0:64, t * 128:(t + 1) * 128],
                        qt_sb[0:64, p, :], start=True, stop=True)
                    nc.tensor.matmul(
                        s_ps[:, u * 512 + 256: u * 512 + 512],
                        kt_o[:, t * 128:(t + 1) * 128],
                        qt_o[:, p, :], start=True, stop=True)
                pt = ptp.tile([128, 1024], dt.float8e5, tag="pt")
                # Q and K both carry x64 -> S is 4096x; exp's scale undoes it
                # and bias -9 recentres P into fp8e5's range (cancels in the
                # softmax numerator/denominator ratio)
                nc.scalar.activation(pt[:], s_ps[:], Exp, scale=2.0 ** -12,
                                     bias=nbias[:])
                return pt

            def emit_av(p, av_ab, g, pt):
                """fp8 DoubleRow AV: each matmul covers both m2 tiles of g."""
                ptu = pt[:].rearrange("q (u x) -> q u x", u=2)
                vr = v_sb[:].rearrange("q (g u h) d -> q g u h d", u=2, h=16)
                for a in range(2):
                    h = 2 * p + a
                    nc.tensor.matmul(
                        av_ab[a][0:65, :],
                        vr[:, g, :, h, :],
                        ptu[:, :, a * 256:(a + 1) * 256],
                        start=(g == 0), stop=(g == 15), perf_mode=DR)

            AV_DELAY = 2  # groups of lag so exp() hides under later QK work

            def emit_pair_finalize(p, av_ab):
                if phase == 2:
                    if sub == "a":
                        return
                    # dump accumulators without the broadcast-divide machinery
                    nc.vector.tensor_copy(ut_sb[0:64, p, :], av_ab[0][0:64, :])
                    st2 = stp.tile([64, N1], dt.bfloat16, tag="st")
                    nc.vector.tensor_copy(st2[:], av_ab[1][0:64, :])
                    nc.sync.dma_start(ut_sb[64:128, p, :], st2[:])
                    return
                # evict undivided U^T halves + denominators first so the AV
                # PSUM banks free before the broadcast-divide chain runs
                ue = bsbp.tile([64, 512], dt.bfloat16, tag="ue")
                nc.vector.tensor_copy(ue[:, 0:256], av_ab[0][0:64, :])
                nc.vector.tensor_copy(ue[:, 256:512], av_ab[1][0:64, :])
                # row 64 of each AV accumulator is the softmax denominator;
                # reciprocal on partition 64, then shift the row to partition 0
                inv = invp.tile([65, 512], dt.bfloat16, tag="inv")
                with nc.allow_low_precision("softmax denom; ~0.4% rel"):
                    nc.vector.reciprocal(inv[64:65, 0:256],
                                         av_ab[0][64:65, :])
                    nc.vector.reciprocal(inv[64:65, 256:512],
                                         av_ab[1][64:65, :])
                nc.sync.dma_start(inv[0:1, :], inv[64:65, :])
                bc_ps = pps.tile([128, 512], dt.float32, tag="qk_ps")
                nc.tensor.matmul(bc_ps[:], ones_b0[:], inv[0:1, :],
                                 start=True, stop=True)
                bc_sb = bsbp.tile([64, 512], dt.float32, tag="bc_sb")
                nc.vector.tensor_copy(bc_sb[:], bc_ps[0:64, :])
                # even head: divide + v_b straight into rows 0:64 of U^T
                nc.vector.tensor_mul(ut_sb[0:64, p, :], ue[:, 0:256],
                                     bc_sb[:, 0:256])
                nc.vector.tensor_scalar_add(
                    ut_sb[0:64, p, :], ut_sb[0:64, p, :],
                    vb_sb[:, 2 * p:2 * p + 1])
                # odd head: staging, then partition-shift DMA to rows 64:128
                st = stp.tile([64, N1], dt.float8e4, tag="st")
                nc.vector.tensor_mul(st[:], ue[:, 256:512],
                                     bc_sb[:, 256:512])
                nc.vector.tensor_scalar_add(st[:], st[:],
                                            vb_sb[:, 2 * p + 1:2 * p + 2])
                nc.sync.dma_start(ut_sb[64:128, p, :], st[:])

            def emit_pair_attn(p, av_ab, kt, kt_o, chunk):
                """Attention for pair p, AV lagging QK/exp by AV_DELAY groups.

                chunk: None = all 16 groups in one go; else g8 index whose
                two groups to emit (interleaved with K-proj of the next pair).
                """
                pend = pend_by_pair.setdefault(p, [])
                groups = range(16) if chunk is None else (2 * chunk,
                                                          2 * chunk + 1)
                for g in groups:
                    pend.append((g, emit_qk_exp(p, kt, kt_o, g)))
                    if len(pend) > AV_DELAY:
                        emit_av(p, av_ab, *pend.pop(0))
                if (chunk is None or chunk == 7) and phase >= 2:
                    for item in pend:
                        emit_av(p, av_ab, *item)
                    pend.clear()
                    if chunk is None:
                        emit_pair_finalize(p, av_ab)
                    else:
                        # defer: the reciprocal+DMA chain runs under the next
                        # pair's first K-block instead of stalling the PE
                        fin_pending.append((p, av_ab))

            def flush_fin():
                while fin_pending:
                    emit_pair_finalize(*fin_pending.pop(0))

            pend_by_pair = {}
            fin_pending = []
            prev = None  # (pair_idx, (av_a, av_b), kt, kt_o)
            for j in range(PAIRS):
                kt = ktp.tile([128, N2], dt.bfloat16, tag="kt")
                kt_o = ktop.tile([64, N2], dt.bfloat16, tag="kt_o")
                kt_tiles.append(kt)
                for g8 in range(8):
                    emit_k_block(j, g8, kt, kt_o)
                    if g8 == 0:
                        flush_fin()
                    if prev is not None and phase >= 2:
                        emit_pair_attn(prev[0], prev[1], prev[2], prev[3], g8)
                av = avp.tile([128, 2 * N1], dt.float32, tag="av")
                av_ab = (av[:, 0:N1], av[:, N1:2 * N1])
                prev = (j, av_ab, kt, kt_o)
            if phase >= 2:
                flush_fin()
                emit_pair_attn(prev[0], prev[1], prev[2], prev[3], None)
            if phase == 1:
                # keep K^T tiles alive / observable: dump slices into O^T
                for co in range(8):
                    nc.vector.tensor_copy(ot_sb[:, co, 0:N1],
                                          kt_tiles[co][:, 0:N1])

        # ---- P projection + L2 normalize (projection pools freed) ----
        with ExitStack() as tail:
            ops = tail.enter_context(
                tc.tile_pool(name="ops", bufs=2, space="PSUM"))
            nps = tail.enter_context(
                tc.tile_pool(name="nps", bufs=1, space="PSUM"))
            sqp = tail.enter_context(tc.tile_pool(name="sqp", bufs=2))
            fop = tail.enter_context(tc.tile_pool(name="fop", bufs=2))

            if phase >= 4:
                for co in range(8):
                    ps = ops.tile([128, N1], dt.float32, tag="o_ps")
                    for cp in range(4):
                        nc.tensor.matmul(
                            ps[:],
                            pw_sb[:, 2 * cp:2 * cp + 2,
                                  co * 128:(co + 1) * 128],
                            ut_sb[:, 2 * cp:2 * cp + 2, :],
                            start=(cp == 0), stop=(cp == 3), perf_mode=DR)
                    nc.vector.tensor_scalar_add(ot_sb[:, co, :], ps[:],
                                                pb_sb[:, co:co + 1])

            if phase >= 5:
                nsq = nps.tile([128, N1], dt.float32, tag="nsq")
                for co in range(8):
                    sq = sqp.tile([128, N1], dt.bfloat16, tag="sq")
                    nc.vector.tensor_mul(sq[:], ot_sb[:, co, :],
                                         ot_sb[:, co, :])
                    nc.tensor.matmul(nsq[0:1, :], ones_bf[:], sq[:],
                                     start=(co == 0), stop=(co == 7))
                lnt = sqp.tile([1, N1], dt.float32, tag="lnt")
                nc.scalar.activation(lnt[:], nsq[0:1, :], Ln)
                invn = sqp.tile([1, N1], dt.bfloat16, tag="invn")
                nc.scalar.activation(invn[:], lnt[:], Exp, scale=-0.5)
                bcn = nps.tile([128, N1], dt.float32, tag="bcn")
                nc.tensor.matmul(bcn[:], ones_b0[:], invn[:],
                                 start=True, stop=True)
                for co in range(8):
                    fo = fop.tile([128, N1], dt.float32, tag="fo")
                    nc.vector.tensor_mul(fo[:], ot_sb[:, co, :], bcn[:])
                    nc.sync.dma_start(ot_d[co * 128:(co + 1) * 128, :], fo[:])
            else:
                for co in range(8):
                    fo = fop.tile([128, N1], dt.float32, tag="fo")
                    nc.vector.tensor_copy(fo[:], ot_sb[:, co, :])
                    nc.sync.dma_start(ot_d[co * 128:(co + 1) * 128, :], fo[:])

    nc.compile()
    return nc


def kernel(E, I, q_w, q_b, k_w, k_b, v_w, v_b, p_w, p_b):
    global _COMPILED, LAST_RESULT
    from concourse import bass_utils

    if _COMPILED is None:
        _COMPILED = _build()
    nc = _COMPILED

    E = np.asarray(E, dtype=np.float32)
    I = np.asarray(I, dtype=np.float32)
    F8 = ml_dtypes.float8_e4m3

    def _wT(w):
        # x64 lifts the (0.02-scale) weights into fp8's normal range; the
        # scale cancels on-device (see _build)
        return np.ascontiguousarray(np.asarray(w, np.float32).T * 64.0
                                    ).astype(F8)

    qw, kw, vw, pw = _wT(q_w), _wT(k_w), _wT(v_w), _wT(p_w)
    qb = np.ascontiguousarray(
        np.asarray(q_b, np.float32).reshape(8, 128).T * 64.0)
    kb = np.ascontiguousarray(
        np.asarray(k_b, np.float32).reshape(8, 128).T * 64.0)
    pb = np.ascontiguousarray(
        np.asarray(p_b, np.float32).reshape(8, 128).T * 64.0)
    vb = np.ascontiguousarray(np.asarray(v_b, np.float32).reshape(16, 64).T)

    in_maps = []
    for b in range(B):
        in_maps.append({
            "it": np.ascontiguousarray(I[b].T).astype(F8),
            "et": np.ascontiguousarray(E[b].T).astype(F8),
            "qw": qw, "kw": kw, "vw": vw, "pw": pw,
            "qb": qb, "kb": kb, "pb": pb, "vb": vb,
        })

    res = bass_utils.run_bass_kernel_spmd(
        nc, in_maps, core_ids=list(range(N_CORES)),
        trace=bool(os.environ.get("BASS_TRACE")))
    LAST_RESULT = res

    out = np.empty((B, N1, 2048), dtype=np.float32)
    for b in range(B):
        out[b, :, :1024] = E[b]
        out[b, :, 1024:] = res.results[b]["ot"].T
    return out



# revision 51
# speedup vs baseline: 1.1279x; 1.0132x over previous
"""Trainium2 Bass kernel for nn_ContextEmbedding (cross-attention context embedding).

Reference math (per batch b):
    Q = E @ q_w.T        [256, 1024]
    K = I @ k_w.T        [4096, 1024]
    V = I @ v_w.T        [4096, 1024]
    S_h = Q_h @ K_h.T    per head (16 heads, head_dim 64)
    P = softmax(S, -1)
    U_h = P_h @ V_h
    O = (U @ p_w.T);  O /= ||O||_2(row)
    out = concat([E, O], -1)   [256, 2048]

Sharding: pure data-parallel over batch B=8 across the 8 NeuronCores (one
batch per core, no collectives). Host pre-transposes/casts activations and
weights to bf16 so every matmul has its contraction dim on SBUF partitions,
and re-assembles the output (E-passthrough concat happens on host).

Per-core dataflow (all matmuls bf16 with f32 PSUM accumulation):
  Q^T [1024,256] and K^T [1024,4096] in o-on-partitions layout, so partition
  tile j holds head pair (2j, 2j+1) in rows 0:64 / 64:128 -> QK^T runs as
  concurrent row-group-tiled matmuls producing S^T [m2, n1]. exp() on ScalarE
  (PSUM->SBUF, 1024-wide ops). V in natural [m2, o] layout with a ones column
  appended per head (65-wide stationary) so AV yields U^T rows 0:64 plus the
  softmax row-sum in row 64. Division by the row-sum uses a ones-matmul
  partition broadcast. P-projection consumes U^T directly; the final L2 norm
  reduces over partitions with a ones-matmul and applies 1/sqrt via
  exp(-0.5*ln(x)) on ScalarE.
"""

import os

import numpy as np
import ml_dtypes

B, N1, N2, D = 8, 256, 4096, 1024
H, HD = 16, 64
PAIRS = H // 2  # 8 partition-tiles of head pairs
N_CORES = 8

BF16 = ml_dtypes.bfloat16

_COMPILED = None  # (nc,) cache so repeated kernel() calls skip the rebuild
LAST_RESULT = None  # BassKernelResults of the most recent run (for harnesses)


def _build():
    import concourse.bacc as bacc
    import concourse.mybir as mybir
    from concourse import tile
    from contextlib import ExitStack

    _ph = os.environ.get("KERNEL_PHASE", "5")
    phase = int(_ph[0])
    sub = _ph[1:]

    dt = mybir.dt
    Exp = mybir.ActivationFunctionType.Exp
    Ln = mybir.ActivationFunctionType.Ln
    DR = mybir.MatmulPerfMode.DoubleRow

    nc = bacc.Bacc("TRN2", target_bir_lowering=False, debug=False,
                   num_devices=N_CORES)

    # ---- per-core DRAM tensors (host pre-transposed / pre-cast) ----
    # Activations and weights are fp8(e4m3); weights carry a x64 scale so
    # their mass sits in fp8's normal range. The scale cancels downstream:
    # exp() applies 2^-12 to Q.K, the softmax divide is scale-invariant
    # (ones column = 64 matches V's x64), and the final L2 norm kills the
    # P-projection's x64. Biases are pre-scaled on host to match.
    it_d = nc.dram_tensor("it", [D, N2], dt.float8e4, kind="ExternalInput")
    et_d = nc.dram_tensor("et", [D, N1], dt.float8e4, kind="ExternalInput")
    qw_d = nc.dram_tensor("qw", [D, D], dt.float8e4, kind="ExternalInput")
    kw_d = nc.dram_tensor("kw", [D, D], dt.float8e4, kind="ExternalInput")
    vw_d = nc.dram_tensor("vw", [D, D], dt.float8e4, kind="ExternalInput")
    pw_d = nc.dram_tensor("pw", [D, D], dt.float8e4, kind="ExternalInput")
    qb_d = nc.dram_tensor("qb", [128, 8], dt.float32, kind="ExternalInput")
    kb_d = nc.dram_tensor("kb", [128, 8], dt.float32, kind="ExternalInput")
    pb_d = nc.dram_tensor("pb", [128, 8], dt.float32, kind="ExternalInput")
    vb_d = nc.dram_tensor("vb", [64, 16], dt.float32, kind="ExternalInput")
    ot_d = nc.dram_tensor("ot", [D, N1], dt.float32, kind="ExternalOutput")

    with tile.TileContext(nc) as tc, ExitStack() as top:
        # ---- long-lived SBUF tiles ----
        persist = top.enter_context(tc.tile_pool(name="persist", bufs=1))
        # Q^T/K^T in fp8: halves QK LDWEIGHTS time via 4-wide FWL. Stored at
        # 32x true scale ((psum + bias)*0.5) so |32Q| stays well under e4m3's
        # 240 max; exp() compensates with scale 2^-10.
        qt_sb = persist.tile([128, PAIRS, N1], dt.float8e4, tag="qt")   # Q^T
        # odd-head halves relocated to partitions 0:64 (operands at SBUF base
        # partition 64 fault the PE on this hardware — re-confirmed)
        qt_o = persist.tile([64, PAIRS, N1], dt.float8e4, tag="qt_o")
        v_sb = persist.tile([128, 512, 65], dt.float8e4, tag="v")       # [V|1]
        ut_sb = persist.tile([128, PAIRS, N1], dt.float8e4, tag="ut")   # U^T
        ot_sb = persist.tile([128, 8, N1], dt.float32, tag="ot")        # O^T
        pw_sb = persist.tile([128, 8, D], dt.float8e4, tag="pw")
        qb_sb = persist.tile([128, 8], dt.float32, tag="qb")
        kb_sb = persist.tile([128, 8], dt.float32, tag="kb")
        pb_sb = persist.tile([128, 8], dt.float32, tag="pb")
        vb_sb = persist.tile([64, 16], dt.float32, tag="vb")
        ones_bf = persist.tile([128, 1], dt.bfloat16, tag="ones_bf")
        ones_f0 = persist.tile([1, 128], dt.float32, tag="ones_f0")
        nbias = persist.tile([128, 1], dt.float32, tag="nbias")
        ones_b0 = persist.tile([1, 128], dt.bfloat16, tag="ones_b0")
        nc.vector.memset(nbias[:], -9.0)
        nc.vector.memset(ones_b0[:], 1.0)

        nc.sync.dma_start(qb_sb[:], qb_d[:])
        nc.sync.dma_start(kb_sb[:], kb_d[:])
        nc.sync.dma_start(pb_sb[:], pb_d[:])
        nc.sync.dma_start(vb_sb[:], vb_d[:])
        if phase < 5:
            nc.vector.memset(ut_sb[:], 0.0)
            nc.vector.memset(ot_sb[:], 0.0)
        nc.vector.memset(ones_bf[:], 1.0)
        nc.vector.memset(ones_f0[:], 1.0)
        # ones column of [V|1]: softmax row-sum lands on PSUM partition 64.
        # V itself carries the weights' x64 scale, so the column is 64 to
        # keep the AV-numerator/denominator ratio at true scale.
        nc.vector.memset(v_sb[:, :, 64:65], 64.0)

        with ExitStack() as proj:
            wpool = proj.enter_context(tc.tile_pool(name="wpool", bufs=1))
            itp = proj.enter_context(tc.tile_pool(name="itp", bufs=1))
            pps = proj.enter_context(
                tc.tile_pool(name="pps", bufs=2, space="PSUM"))

            kw_sb = wpool.tile([128, 8, D], dt.float8e4, tag="kw")
            it_sb = itp.tile([128, 8, N2], dt.float8e4, tag="it")

            def emit_it_chunk(ch):
                for c in range(8):
                    nc.sync.dma_start(
                        it_sb[:, c, ch * 1024:(ch + 1) * 1024],
                        it_d[c * 128:(c + 1) * 128, ch * 1024:(ch + 1) * 1024])

            with ExitStack() as qscope:
                qep = qscope.enter_context(tc.tile_pool(name="qep", bufs=1))
                et_sb = qep.tile([128, 8, N1], dt.float8e4, tag="et")
                qw_sb = qep.tile([128, 8, D], dt.float8e4, tag="qw")
                for c in range(8):
                    nc.sync.dma_start(et_sb[:, c, :],
                                      et_d[c * 128:(c + 1) * 128, :])
                    nc.sync.dma_start(qw_sb[:, c, :],
                                      qw_d[c * 128:(c + 1) * 128, :])
                emit_it_chunk(0)
                # ---- Q^T projection (fp8 DoubleRow: c-pairs of k-subtiles) ----
                for j in range(PAIRS):
                    ps = pps.tile([128, 512], dt.float32, tag="qk_ps")
                    for cp in range(4):
                        nc.tensor.matmul(
                            ps[:, 0:N1],
                            qw_sb[:, 2 * cp:2 * cp + 2, j * 128:(j + 1) * 128],
                            et_sb[:, 2 * cp:2 * cp + 2, :],
                            start=(cp == 0), stop=(cp == 3), perf_mode=DR)
                    nc.vector.tensor_scalar(qt_sb[:, j, :], ps[:, 0:N1],
                                            qb_sb[:, j:j + 1], 0.5,
                                            mybir.AluOpType.add,
                                            mybir.AluOpType.mult)
                    nc.sync.dma_start(qt_o[:, j, :], qt_sb[64:128, j, :])

            # ---- V projection (natural layout, strided into [V|1] slots) ----
            with ExitStack() as vscope:
                vwp = vscope.enter_context(tc.tile_pool(name="vwp", bufs=1))
                vw_sb = vwp.tile([128, 8, D], dt.float8e4, tag="vw")
                for c in range(8):
                    nc.sync.dma_start(vw_sb[:, c, :],
                                      vw_d[c * 128:(c + 1) * 128, :])
                for ch in range(1, 4):
                    emit_it_chunk(ch)
                # kw/pw after the it chunks: they are needed later and must
                # not delay the V-projection's input stream
                for c in range(8):
                    nc.sync.dma_start(kw_sb[:, c, :],
                                      kw_d[c * 128:(c + 1) * 128, :])
                for c in range(8):
                    nc.sync.dma_start(pw_sb[:, c, :],
                                      pw_d[c * 128:(c + 1) * 128, :])
                vps = vscope.enter_context(
                    tc.tile_pool(name="vps", bufs=3, space="PSUM"))
                for t in range(32):
                    # the it-chunk stationary is reused by both output halves
                    ps0 = vps.tile([128, 512], dt.float32, tag="v_ps")
                    ps1 = vps.tile([128, 512], dt.float32, tag="v_ps")
                    pss = (ps0, ps1)
                    for cp in range(4):
                        for s in range(2):
                            nc.tensor.matmul(
                                pss[s][:],
                                it_sb[:, 2 * cp:2 * cp + 2,
                                      t * 128:(t + 1) * 128],
                                vw_sb[:, 2 * cp:2 * cp + 2,
                                      s * 512:(s + 1) * 512],
                                start=(cp == 0), stop=(cp == 3), perf_mode=DR)
                    for s in range(2):
                        dst = v_sb[:, t * 16 + s * 8: t * 16 + s * 8 + 8, 0:64]
                        nc.vector.tensor_copy(dst, pss[s][:].rearrange(
                            "p (h d) -> p h d", d=64))

            # ---- K^T projection interleaved with attention ----
            ktp = proj.enter_context(tc.tile_pool(name="ktp", bufs=2))
            ktop = proj.enter_context(tc.tile_pool(name="ktop", bufs=2))
            sps = proj.enter_context(
                tc.tile_pool(name="sps", bufs=2, space="PSUM"))
            avp = proj.enter_context(
                tc.tile_pool(name="avp", bufs=2, space="PSUM"))
            ptp = proj.enter_context(tc.tile_pool(name="ptp", bufs=4))
            invp = proj.enter_context(tc.tile_pool(name="invp", bufs=1))
            bsbp = proj.enter_context(tc.tile_pool(name="bsbp", bufs=1))
            stp = proj.enter_context(tc.tile_pool(name="stp", bufs=1))

            kt_tiles = []

            def emit_k_block(j, g8, kt, kt_o):
                ps = pps.tile([128, 512], dt.float32, tag="qk_ps")
                for cp in range(4):
                    nc.tensor.matmul(
                        ps[:],
                        kw_sb[:, 2 * cp:2 * cp + 2, j * 128:(j + 1) * 128],
                        it_sb[:, 2 * cp:2 * cp + 2,
                              g8 * 512:(g8 + 1) * 512],
                        start=(cp == 0), stop=(cp == 3), perf_mode=DR)
                nc.vector.tensor_scalar(
                    kt[:, g8 * 512:(g8 + 1) * 512], ps[:],
                    kb_sb[:, j:j + 1], 0.5,
                    mybir.AluOpType.add, mybir.AluOpType.mult)
                nc.sync.dma_start(kt_o[:, g8 * 512:(g8 + 1) * 512],
                                  kt[64:128, g8 * 512:(g8 + 1) * 512])

            def emit_qk_exp(p, kt, kt_o, g):
                """QK^T + exp for group g (m2 tiles 2g, 2g+1); returns pt."""
                s_ps = sps.tile([128, 1024], dt.float32, tag="s_ps")
                for u in range(2):
                    t = 2 * g + u
                    nc.tensor.matmul(
                        s_ps[:, u * 512: u * 512 + 256],
                        kt[0:64, t * 128:(t + 1) * 128],
                        qt_sb[0:64, p, :], start=True, stop=True)
                    nc.tensor.matmul(
                        s_ps[:, u * 512 + 256: u * 512 + 512],
                        kt_o[:, t * 128:(t + 1) * 128],
                        qt_o[:, p, :], start=True, stop=True)
                pt = ptp.tile([128, 1024], dt.float8e5, tag="pt")
                # Q and K both carry x64 -> S is 4096x; exp's scale undoes it
                # and bias -9 recentres P into fp8e5's range (cancels in the
                # softmax numerator/denominator ratio)
                nc.scalar.activation(pt[:], s_ps[:], Exp, scale=2.0 ** -10,
                                     bias=nbias[:])
                return pt

            def emit_av(p, av_ab, g, pt):
                """fp8 DoubleRow AV: each matmul covers both m2 tiles of g."""
                ptu = pt[:].rearrange("q (u x) -> q u x", u=2)
                vr = v_sb[:].rearrange("q (g u h) d -> q g u h d", u=2, h=16)
                for a in range(2):
                    h = 2 * p + a
                    nc.tensor.matmul(
                        av_ab[a][0:65, :],
                        vr[:, g, :, h, :],
                        ptu[:, :, a * 256:(a + 1) * 256],
                        start=(g == 0), stop=(g == 15), perf_mode=DR)

            AV_DELAY = 3  # groups of lag so exp() hides under later QK work

            def emit_pair_finalize(p, av_ab):
                if phase == 2:
                    if sub == "a":
                        return
                    # dump accumulators without the broadcast-divide machinery
                    nc.vector.tensor_copy(ut_sb[0:64, p, :], av_ab[0][0:64, :])
                    st2 = stp.tile([64, N1], dt.bfloat16, tag="st")
                    nc.vector.tensor_copy(st2[:], av_ab[1][0:64, :])
                    nc.sync.dma_start(ut_sb[64:128, p, :], st2[:])
                    return
                # evict undivided U^T halves + denominators first so the AV
                # PSUM banks free before the broadcast-divide chain runs
                ue = bsbp.tile([64, 512], dt.bfloat16, tag="ue")
                nc.vector.tensor_copy(ue[:, 0:256], av_ab[0][0:64, :])
                nc.vector.tensor_copy(ue[:, 256:512], av_ab[1][0:64, :])
                # row 64 of each AV accumulator is the softmax denominator;
                # reciprocal on partition 64, then shift the row to partition 0
                inv = invp.tile([65, 512], dt.bfloat16, tag="inv")
                with nc.allow_low_precision("softmax denom; ~0.4% rel"):
                    nc.vector.reciprocal(inv[64:65, 0:256],
                                         av_ab[0][64:65, :])
                    nc.vector.reciprocal(inv[64:65, 256:512],
                                         av_ab[1][64:65, :])
                nc.sync.dma_start(inv[0:1, :], inv[64:65, :])
                bc_ps = pps.tile([128, 512], dt.float32, tag="qk_ps")
                nc.tensor.matmul(bc_ps[:], ones_b0[:], inv[0:1, :],
                                 start=True, stop=True)
                bc_sb = bsbp.tile([64, 512], dt.float32, tag="bc_sb")
                nc.vector.tensor_copy(bc_sb[:], bc_ps[0:64, :])
                # even head: divide + v_b straight into rows 0:64 of U^T
                nc.vector.tensor_mul(ut_sb[0:64, p, :], ue[:, 0:256],
                                     bc_sb[:, 0:256])
                nc.vector.tensor_scalar_add(
                    ut_sb[0:64, p, :], ut_sb[0:64, p, :],
                    vb_sb[:, 2 * p:2 * p + 1])
                # odd head: staging, then partition-shift DMA to rows 64:128
                st = stp.tile([64, N1], dt.float8e4, tag="st")
                nc.vector.tensor_mul(st[:], ue[:, 256:512],
                                     bc_sb[:, 256:512])
                nc.vector.tensor_scalar_add(st[:], st[:],
                                            vb_sb[:, 2 * p + 1:2 * p + 2])
                nc.sync.dma_start(ut_sb[64:128, p, :], st[:])

            def emit_pair_attn(p, av_ab, kt, kt_o, chunk):
                """Attention for pair p, AV lagging QK/exp by AV_DELAY groups.

                chunk: None = all 16 groups in one go; else g8 index whose
                two groups to emit (interleaved with K-proj of the next pair).
                """
                pend = pend_by_pair.setdefault(p, [])
                groups = range(16) if chunk is None else (2 * chunk,
                                                          2 * chunk + 1)
                for g in groups:
                    pend.append((g, emit_qk_exp(p, kt, kt_o, g)))
                    if len(pend) > AV_DELAY:
                        emit_av(p, av_ab, *pend.pop(0))
                if (chunk is None or chunk == 7) and phase >= 2:
                    for item in pend:
                        emit_av(p, av_ab, *item)
                    pend.clear()
                    if chunk is None:
                        emit_pair_finalize(p, av_ab)
                    else:
                        # defer: the reciprocal+DMA chain runs under the next
                        # pair's first K-block instead of stalling the PE
                        fin_pending.append((p, av_ab))

            def flush_fin():
                while fin_pending:
                    emit_pair_finalize(*fin_pending.pop(0))

            pend_by_pair = {}
            fin_pending = []
            prev = None  # (pair_idx, (av_a, av_b), kt, kt_o)
            for j in range(PAIRS):
                kt = ktp.tile([128, N2], dt.float8e4, tag="kt")
                kt_o = ktop.tile([64, N2], dt.float8e4, tag="kt_o")
                kt_tiles.append(kt)
                for g8 in range(8):
                    emit_k_block(j, g8, kt, kt_o)
                    if g8 == 0:
                        flush_fin()
                    if prev is not None and phase >= 2:
                        emit_pair_attn(prev[0], prev[1], prev[2], prev[3], g8)
                av = avp.tile([128, 2 * N1], dt.float32, tag="av")
                av_ab = (av[:, 0:N1], av[:, N1:2 * N1])
                prev = (j, av_ab, kt, kt_o)
            if phase >= 2:
                flush_fin()
                emit_pair_attn(prev[0], prev[1], prev[2], prev[3], None)
            if phase == 1:
                # keep K^T tiles alive / observable: dump slices into O^T
                for co in range(8):
                    nc.vector.tensor_copy(ot_sb[:, co, 0:N1],
                                          kt_tiles[co][:, 0:N1])

        # ---- P projection + L2 normalize (projection pools freed) ----
        with ExitStack() as tail:
            ops = tail.enter_context(
                tc.tile_pool(name="ops", bufs=2, space="PSUM"))
            nps = tail.enter_context(
                tc.tile_pool(name="nps", bufs=1, space="PSUM"))
            sqp = tail.enter_context(tc.tile_pool(name="sqp", bufs=2))
            fop = tail.enter_context(tc.tile_pool(name="fop", bufs=2))

            nsq = nps.tile([128, N1], dt.float32, tag="nsq")
            if phase >= 4:
                for co in range(8):
                    ps = ops.tile([128, N1], dt.float32, tag="o_ps")
                    for cp in range(4):
                        nc.tensor.matmul(
                            ps[:],
                            pw_sb[:, 2 * cp:2 * cp + 2,
                                  co * 128:(co + 1) * 128],
                            ut_sb[:, 2 * cp:2 * cp + 2, :],
                            start=(cp == 0), stop=(cp == 3), perf_mode=DR)
                    nc.vector.tensor_scalar_add(ot_sb[:, co, :], ps[:],
                                                pb_sb[:, co:co + 1])
                    if phase >= 5:
                        # norm reduction rides along with the projection so
                        # the tail chain only has Ln/Exp/broadcast left
                        sq = sqp.tile([128, N1], dt.bfloat16, tag="sq")
                        nc.vector.tensor_mul(sq[:], ot_sb[:, co, :],
                                             ot_sb[:, co, :])
                        nc.tensor.matmul(nsq[0:1, :], ones_bf[:], sq[:],
                                         start=(co == 0), stop=(co == 7))

            if phase >= 5:
                lnt = sqp.tile([1, N1], dt.float32, tag="lnt")
                nc.scalar.activation(lnt[:], nsq[0:1, :], Ln)
                invn = sqp.tile([1, N1], dt.bfloat16, tag="invn")
                nc.scalar.activation(invn[:], lnt[:], Exp, scale=-0.5)
                bcn = nps.tile([128, N1], dt.float32, tag="bcn")
                nc.tensor.matmul(bcn[:], ones_b0[:], invn[:],
                                 start=True, stop=True)
                for co in range(8):
                    fo = fop.tile([128, N1], dt.float32, tag="fo")
                    nc.vector.tensor_mul(fo[:], ot_sb[:, co, :], bcn[:])
                    nc.sync.dma_start(ot_d[co * 128:(co + 1) * 128, :], fo[:])
            else:
                for co in range(8):
                    fo = fop.tile([128, N1], dt.float32, tag="fo")
                    nc.vector.tensor_copy(fo[:], ot_sb[:, co, :])
                    nc.sync.dma_start(ot_d[co * 128:(co + 1) * 128, :], fo[:])

    nc.compile()
    return nc


def kernel(E, I, q_w, q_b, k_w, k_b, v_w, v_b, p_w, p_b):
    global _COMPILED, LAST_RESULT
    from concourse import bass_utils

    if _COMPILED is None:
        _COMPILED = _build()
    nc = _COMPILED

    E = np.asarray(E, dtype=np.float32)
    I = np.asarray(I, dtype=np.float32)
    F8 = ml_dtypes.float8_e4m3

    def _wT(w):
        # x64 lifts the (0.02-scale) weights into fp8's normal range; the
        # scale cancels on-device (see _build)
        return np.ascontiguousarray(np.asarray(w, np.float32).T * 64.0
                                    ).astype(F8)

    qw, kw, vw, pw = _wT(q_w), _wT(k_w), _wT(v_w), _wT(p_w)
    qb = np.ascontiguousarray(
        np.asarray(q_b, np.float32).reshape(8, 128).T * 64.0)
    kb = np.ascontiguousarray(
        np.asarray(k_b, np.float32).reshape(8, 128).T * 64.0)
    pb = np.ascontiguousarray(
        np.asarray(p_b, np.float32).reshape(8, 128).T * 64.0)
    vb = np.ascontiguousarray(np.asarray(v_b, np.float32).reshape(16, 64).T)

    in_maps = []
    for b in range(B):
        in_maps.append({
            "it": np.ascontiguousarray(I[b].T).astype(F8),
            "et": np.ascontiguousarray(E[b].T).astype(F8),
            "qw": qw, "kw": kw, "vw": vw, "pw": pw,
            "qb": qb, "kb": kb, "pb": pb, "vb": vb,
        })

    res = bass_utils.run_bass_kernel_spmd(
        nc, in_maps, core_ids=list(range(N_CORES)),
        trace=bool(os.environ.get("BASS_TRACE")))
    LAST_RESULT = res

    out = np.empty((B, N1, 2048), dtype=np.float32)
    for b in range(B):
        out[b, :, :1024] = E[b]
        out[b, :, 1024:] = res.results[b]["ot"].T
    return out

